# revision 15
# baseline (speedup 1.0000x reference)
"""Causal GQA attention block (RoPE, 32 q-heads / 8 kv-heads, fp32 I/O) on
8 Trainium2 NeuronCores.  Final: engine-rebalanced + DMA-consolidated
(HW-measured ~542us vs the 967us baseline, rel err 3.9e-3).

Sharding: sequence-parallel (unchanged from baseline). Core c owns batch
b = c//4 and query blocks {j, 7-j}, j = c%4; k/v all-gathered within the
4-core batch group; attention + out-proj fully local; host concatenates.

v2 changes vs baseline (engine-time budget per the CoreSim cost model):
- RoPE: instead of 12 narrow [32,512] DVE ops per 128-row tile (131us
  DVE total), drain the projection psum twice on ACT (straight copy +
  32-row-swapped copy, ACT is idle during projections) and rotate with
  3 full-width bf16 DVE ops against host-built cosR=[c;c;c;c] /
  sinR=[-s;s;-s;s] tiles: out = tn*cosR + tsw*sinR.  DVE 131us -> 20us.
- exp: one ACT op per (kv,g,pair) [128,1024] for causally-live tiles;
  for g>=8 the two heads' scores land compactly in one [128,512] psum
  region -> one exp per (kv,g) over [128,1024] for all 4 heads.
- masks: host ships mask2[g] = [maskA_g | ones] (g<8, B-half of every
  core is fully causal there) or [maskB_g | maskB_g] (g>=8); one
  tensor-tensor multiply per exp op, split between DVE and GpSimd.
- softmax denominators: ones-column in v (unchanged); normalization per
  kv group: DVE copies of the sum rows, DVE reciprocals, K=1 broadcast
  matmuls on PE (tile-positioned), DVE muls into the y tile.
- DMA consolidation (the big HW win, ~200us): weight tiles arrive in
  one multi-level-AP DMA per projection column block (and one 2MB DMA
  per out-proj block); gathered k/v are staged once into SBUF-resident
  tiles (kdup duplicates each kv head into both 64-row halves to feed
  the row-tiled score matmul pairs; vres carries a ones column per
  (g,kv) block), replacing 384 small per-iteration DMAs (~2us fixed
  cost each); the k and v all-gathers are split so the k-gather
  overlaps the v and q projections.
- out-proj: loop order (oc, ct, tt) so each Wo tile is DMA'd once
  (baseline re-loaded Wo 4x = 25MB extra HBM traffic).

SPMD note: all 8 cores share one program; per-core causal variation is
handled entirely by the host-built masks (diagonal tiles + beyond-extent
tiles multiply to zero).
"""

import sys
import json

sys.path.insert(0, "/opt/trn_rl_repo")

import numpy as np
import ml_dtypes

import concourse.bass as bass
import concourse.tile as tile
from concourse import mybir

F32 = mybir.dt.float32
BF16 = mybir.dt.bfloat16
BF = ml_dtypes.bfloat16
AF = mybir.ActivationFunctionType

# ---------------------------------------------------------------------------
# walrus workaround: this build supports one semaphore wait per instruction,
# but TileContext's tail drain attaches several. Split the extras onto
# standalone EventSemaphore instructions placed just before the instruction.
# ---------------------------------------------------------------------------


def _fix_multiwait(bir_bytes):
    d = json.loads(bir_bytes)
    ctr = 0
    changed = False
    for fn in d.get("functions", []):
        for blk in fn.get("blocks", []):
            new_insts = []
            for inst in blk["instructions"]:
                si = inst.get("sync_info") or {}
                waits = si.get("on_wait") or []
                if len(waits) > 1:
                    changed = True
                    for w in waits[:-1]:
                        ctr += 1
                        new_insts.append({
                            "debug": inst.get("debug", 0),
                            "engine": inst["engine"],
                            "ins": [],
                            "name": f"mwfix_{ctr}_{inst['name']}",
                            "opcode": "EventSemaphore",
                            "outs": [],
                            "sync_info": {"on_update": [], "on_wait": [w]},
                        })
                    si["on_wait"] = [waits[-1]]
                new_insts.append(inst)
            blk["instructions"] = new_insts
    return json.dumps(d).encode() if changed else bir_bytes


def _install_birfix():
    from concourse import bass_utils, bass2jax

    if getattr(bass_utils, "_mwfix_installed", False):
        return
    orig = bass_utils.compile_bir_kernel

    def patched(bir_json, tmpdir, neff_name="file.neff", **kw):
        if isinstance(bir_json, str):
            bir_json = bir_json.encode()
        return orig(_fix_multiwait(bir_json), tmpdir, neff_name, **kw)

    bass_utils.compile_bir_kernel = patched
    bass_utils._mwfix_installed = True
    bass2jax.compile_bir_kernel = patched


# ---------------------------------------------------------------------------
# configuration
# ---------------------------------------------------------------------------


class Cfg:
    def __init__(self, B=2, T=2048, DIM=2048, NH=32, NKV=8, HD=64,
                 rope_base=10000.0):
        self.B, self.T, self.DIM = B, T, DIM
        self.NH, self.NKV, self.HD = NH, NKV, HD
        self.rope_base = rope_base
        self.NCORES = 8
        self.BLK = T // 8            # tokens per query block
        self.KT = self.BLK // 2      # tokens per k-tile (partition dim)
        self.TOK = 2 * self.BLK      # tokens per core
        self.KDIM = NKV * HD
        self.GQ = NH // NKV          # q heads per kv head (4)
        self.NKT = 16                # k-tiles in a full sequence
        self.NCT = DIM // 128        # contraction tiles over model dim
        self.HD2 = HD // 2


FULL = Cfg()


def core_blocks(c):
    return c // 4, c % 4, 7 - (c % 4)


def ktile_src(cfg, g):
    """k-tile g (tokens [g*KT,(g+1)*KT)) -> (owner group-slot, col base)."""
    i = g // 2
    jj = min(i, 7 - i)
    colbase = (0 if i == jj else cfg.BLK) + (g % 2) * cfg.KT
    return jj, colbase


# ---------------------------------------------------------------------------
# device program
# ---------------------------------------------------------------------------


def build_nc(cfg: Cfg, reps=1, ablate=()):
    c = cfg
    ab = set(ablate)
    nc = bass.Bass(num_devices=c.NCORES)

    xT = nc.declare_dram_parameter("xT", [c.DIM, c.TOK], BF16, isOutput=False)
    wqT = nc.declare_dram_parameter("wqT", [c.DIM, c.DIM], BF16, isOutput=False)
    wkT = nc.declare_dram_parameter("wkT", [c.DIM, c.KDIM], BF16, isOutput=False)
    wvT = nc.declare_dram_parameter("wvT", [c.DIM, c.KDIM], BF16, isOutput=False)
    woT = nc.declare_dram_parameter("woT", [c.DIM, c.DIM], BF16, isOutput=False)
    cosR = nc.declare_dram_parameter("cosR", [128, c.TOK], BF16, isOutput=False)
    sinR = nc.declare_dram_parameter("sinR", [128, c.TOK], BF16, isOutput=False)
    masks = nc.declare_dram_parameter("masks", [c.KT, c.NKT * c.TOK], BF16,
                                      isOutput=False)
    out = nc.declare_dram_parameter("out", [c.TOK, c.DIM], F32, isOutput=True)

    k_bounce = nc.dram_tensor("k_bounce", [c.KDIM * c.TOK], BF16)
    k_all = nc.dram_tensor("k_all", [4, c.KDIM * c.TOK], BF16)
    v_bounce = nc.dram_tensor("v_bounce", [c.KDIM * c.TOK], BF16)
    v_all = nc.dram_tensor("v_all", [4, c.KDIM * c.TOK], BF16)

    def k_view(ap):       # feature-major [KDIM, TOK]
        return ap.rearrange("(f t) -> f t", t=c.TOK)

    def v_view(ap):       # token-major [TOK, KDIM]
        return ap.rearrange("(t f) -> t f", f=c.KDIM)

    with tile.TileContext(nc) as tc:
        with tc.tile_pool(name="persist", bufs=1) as persist:
            # x^T resident: chunk ct (features [128ct,+128)) at cols [TOK*ct]
            xT_sb = persist.tile([128, c.NCT * c.TOK], BF16)
            for ct in range(c.NCT):
                nc.sync.dma_start(
                    xT_sb[:, ct * c.TOK:(ct + 1) * c.TOK],
                    xT[ct * 128:(ct + 1) * 128, :])
            cos_sb = persist.tile([128, c.TOK], BF16)
            sin_sb = persist.tile([128, c.TOK], BF16)
            nc.sync.dma_start(cos_sb[:], cosR[:])
            nc.sync.dma_start(sin_sb[:], sinR[:])
            mask_sb = persist.tile([c.KT, c.NKT * c.TOK], BF16)
            nc.sync.dma_start(mask_sb[:], masks[:])
            qT_sb = persist.tile([128, (c.NH // 2) * c.TOK], BF16)
            yT_sb = persist.tile([128, c.NCT * c.TOK], BF16)
            kT_sb = persist.tile([128, (c.KDIM // 128) * c.TOK], BF16)
            v_sb = persist.tile([128, (c.TOK // 128) * c.KDIM], BF16)
            # gathered k/v resident in SBUF for the whole attention phase:
            # kdup: per kv head, full-sequence k duplicated into both 64-row
            # halves (feeds the row-tiled score matmul pairs directly);
            # vres: per (k-tile g, kv) a [128,65] block = 64 v features plus
            # a ones column (softmax denominator comes from the AV matmul).
            kdup = persist.tile([128, c.NKV * c.T], BF16)
            vres = persist.tile([128, c.NKT * c.NKV * (c.HD + 1)], BF16)
            # K=1 broadcast lhsT rows at base partitions 0/32/64/96 (matmul
            # operands must start at a 32-aligned partition).
            ones4 = persist.tile([97, c.HD], F32)
            nc.vector.memset(ones4[:], 1.0)

            def xt_chunk(ct):
                return xT_sb[:, ct * c.TOK:(ct + 1) * c.TOK]

            def yhead_ap(h):
                # head h's 64 feature rows inside yT's [128, NCT*TOK] layout
                a, r = h // 2, (h % 2) * c.HD
                return yT_sb[r:r + c.HD, a * c.TOK:(a + 1) * c.TOK]

            # ---------------- projections + rope ----------------
            for _rep in range(reps):
                with tc.tile_pool(name="wpool", bufs=3) as wpool, \
                   tc.tile_pool(name="pspool", bufs=4, space="PSUM") as pspool, \
                   tc.tile_pool(name="dvetmp", bufs=8) as dvetmp:

                  def proj_tile(wT_h, ot):
                      """psum [128, TOK] = output-feature rows [128ot,+128).
                      The 16 per-ct weight chunks arrive in ONE 512KB DMA
                      (multi-level source AP over the row blocks) instead of
                      16 separate 32KB transfers."""
                      ps = pspool.tile([128, c.TOK], F32, tag="proj")
                      w_sb = wpool.tile([128, c.NCT * 128], BF16, tag="w")
                      nc.sync.dma_start(
                          w_sb[:].rearrange("p (ct o) -> p ct o", ct=c.NCT),
                          wT_h[:, ot * 128:(ot + 1) * 128].rearrange(
                              "(ct p) o -> p ct o", p=128))
                      for ct in range(c.NCT):
                          nc.tensor.matmul(ps[:],
                                           w_sb[:, ct * 128:(ct + 1) * 128],
                                           xt_chunk(ct),
                                           start=(ct == 0), stop=(ct == c.NCT - 1))
                      return ps

                  def rope_tile(ps, dst):
                      """ps rows = [ev0;od0;ev1;od1] (2 heads x 32).  Rotate
                      into dst [128, TOK]: out = tn*cosR + tsw*sinR where
                      tn = ps, tsw = 32-row-swapped ps, cosR=[c;c;c;c],
                      sinR=[-s;s;-s;s].  Drains on ACT (idle in this phase),
                      arithmetic on DVE in full-width bf16 2x ops."""
                      if "rope" in ab:
                          return
                      tn = dvetmp.tile([128, c.TOK], BF16, tag="tn")
                      nc.scalar.copy(tn[:], ps[:])
                      tsw = dvetmp.tile([128, c.TOK], BF16, tag="tsw")
                      for b in range(4):
                          nc.scalar.copy(
                              tsw[(b ^ 1) * 32:((b ^ 1) + 1) * 32, :],
                              ps[b * 32:(b + 1) * 32, :])
                      m1 = dvetmp.tile([128, c.TOK], BF16, tag="m1")
                      nc.vector.tensor_mul(m1[:], tn[:], cos_sb[:])
                      m2 = dvetmp.tile([128, c.TOK], BF16, tag="m2")
                      nc.vector.tensor_mul(m2[:], tsw[:], sin_sb[:])
                      nc.vector.tensor_add(dst, m1[:], m2[:])

                  # k projection (feature-major) + rope -> bounce; the
                  # k-gather launches as soon as k is written, so it overlaps
                  # the v and q projections (v has its own gather below).
                  for ot in range(c.KDIM // 128):
                      ps = proj_tile(wkT, ot)
                      rope_tile(ps, kT_sb[:, ot * c.TOK:(ot + 1) * c.TOK])
                  for ot in range(c.KDIM // 128):
                      nc.sync.dma_start(
                          k_view(k_bounce)[ot * 128:(ot + 1) * 128, :],
                          kT_sb[:, ot * c.TOK:(ot + 1) * c.TOK])
                  if "gather" not in ab:
                      nc.gpsimd.collective_compute(
                          "AllGather", mybir.AluOpType.bypass,
                          replica_groups=[[0, 1, 2, 3], [4, 5, 6, 7]],
                          ins=[k_bounce[:]], outs=[k_all[:]])

                  # v projection (token-major): v[t,f] tiles via lhsT = x^T
                  psvs = [pspool.tile([128, c.KDIM], F32, tag="projv",
                                      name=f"psv_{_rep}_{i}")
                          for i in range(c.TOK // 128)]
                  for ct in range(c.NCT):
                      wv_sb = wpool.tile([128, c.KDIM], BF16, tag="wv")
                      nc.sync.dma_start(
                          wv_sb[:], wvT[ct * 128:(ct + 1) * 128, :])
                      for tt in range(c.TOK // 128):
                          nc.tensor.matmul(
                              psvs[tt][:],
                              xt_chunk(ct)[:, tt * 128:(tt + 1) * 128],
                              wv_sb[:], start=(ct == 0), stop=(ct == c.NCT - 1))
                  for tt in range(c.TOK // 128):
                      nc.vector.tensor_copy(
                          v_sb[:, tt * c.KDIM:(tt + 1) * c.KDIM], psvs[tt][:])
                  for tt in range(c.TOK // 128):
                      nc.sync.dma_start(
                          v_view(v_bounce)[tt * 128:(tt + 1) * 128, :],
                          v_sb[:, tt * c.KDIM:(tt + 1) * c.KDIM])
                  if "gather" not in ab:
                      nc.gpsimd.collective_compute(
                          "AllGather", mybir.AluOpType.bypass,
                          replica_groups=[[0, 1, 2, 3], [4, 5, 6, 7]],
                          ins=[v_bounce[:]], outs=[v_all[:]])

                  # q projection + rope (overlaps the gather).
                  # pair p = feature chunk ot: head 2p at rows [0:64],
                  # head 2p+1 at rows [64:128] (row-tiling layout).
                  for ot in range(c.NCT):
                      ps = proj_tile(wqT, ot)
                      rope_tile(ps, qT_sb[:, ot * c.TOK:(ot + 1) * c.TOK])

                # ---------------- attention ----------------
                with tc.tile_pool(name="spool", bufs=2, space="PSUM") as spool, \
                     tc.tile_pool(name="avpool", bufs=4, space="PSUM") as avpool, \
                     tc.tile_pool(name="epool", bufs=10) as epool, \
                     tc.tile_pool(name="npool", bufs=3) as npool:

                    skip_attn = "attn" in ab
                    mask_eng = [nc.vector, nc.gpsimd]

                    # Stage gathered k/v into the resident SBUF tiles.
                    # kdup[kv]: seq-ordered k, duplicated into both 64-row
                    # halves.  Slot jj of k_all holds seq blocks jj and 7-jj
                    # (256 tokens each, consecutive in the slot), so one DMA
                    # per (jj, row-half) covers all 8 kv heads x both blocks
                    # via 4-level access patterns.
                    nc.vector.memset(vres[:], 1.0)
                    VB = c.HD + 1
                    for jj in range(4):
                        for half in range(2):
                            r0 = half * c.HD
                            src = k_view(k_all[jj]).rearrange(
                                "(kv u) t -> u kv t", u=c.HD)
                            for bpos, scol in ((jj, 0), (7 - jj, c.BLK)):
                                nc.sync.dma_start(
                                    kdup[r0:r0 + c.HD, :].rearrange(
                                        "p (kv s) -> p kv s", kv=c.NKV)[
                                        :, :, bpos * c.BLK:(bpos + 1) * c.BLK],
                                    src[:, :, scol:scol + c.BLK])
                    for g in range(c.NKT):
                        jj, colbase = ktile_src(c, g)
                        nc.sync.dma_start(
                            vres[:, g * c.NKV * VB:(g + 1) * c.NKV * VB]
                            .rearrange("p (kv f) -> p kv f", kv=c.NKV)[
                                :, :, 0:c.HD],
                            v_view(v_all[jj])[colbase:colbase + c.KT, :]
                            .rearrange("p (kv f) -> p kv f", kv=c.NKV))

                    def kd(half, kv, g):
                        r0 = half * c.HD
                        col = kv * c.T + g * c.KT
                        return kdup[r0:r0 + c.HD, col:col + c.KT]

                    def vt(kv, g):
                        col = (g * c.NKV + kv) * VB
                        return vres[:, col:col + VB]

                    for kv in range(c.NKV if not skip_attn else 0):
                        av_ps = [avpool.tile([c.HD + 1, c.TOK], F32, tag="av",
                                             name=f"av_{_rep}_{kv}_{i}")
                                 for i in range(c.GQ)]
                        for g in range(c.NKT):
                            mk = mask_sb[:, g * c.TOK:(g + 1) * c.TOK]
                            a_live = g < c.NKT // 2
                            if a_live:
                                # two [128,1024] lanes, one per head pair
                                for p in range(c.GQ // 2):
                                    h0 = kv * c.GQ + 2 * p
                                    pcol = (h0 // 2) * c.TOK
                                    sps = spool.tile([c.KT, 2 * c.TOK], F32,
                                                     tag="s")
                                    nc.tensor.matmul(
                                        sps[:, 0:c.TOK], kd(0, kv, g),
                                        qT_sb[0:c.HD, pcol:pcol + c.TOK],
                                        start=True, stop=True)
                                    nc.tensor.matmul(
                                        sps[:, c.TOK:2 * c.TOK],
                                        kd(1, kv, g),
                                        qT_sb[c.HD:2 * c.HD, pcol:pcol + c.TOK],
                                        start=True, stop=True)
                                    ex = epool.tile([c.KT, 2 * c.TOK], BF16,
                                                    tag="ex")
                                    if "exp" in ab:
                                        nc.scalar.activation(
                                            ex[0:1, 0:2], sps[0:1, 0:2], AF.Exp,
                                            bias=0.0, scale=0.125)
                                    else:
                                        nc.scalar.activation(
                                            ex[:], sps[:], AF.Exp, bias=0.0,
                                            scale=float(1.0 / np.sqrt(c.HD)))
                                    if "mask" in ab:
                                        exm = ex
                                    else:
                                        exm = epool.tile([c.KT, 2 * c.TOK],
                                                         BF16, tag="exm")
                                        for hh in range(2):
                                            mask_eng[hh].tensor_mul(
                                                exm[:, hh * c.TOK:
                                                    (hh + 1) * c.TOK],
                                                ex[:, hh * c.TOK:
                                                   (hh + 1) * c.TOK], mk)
                                    for hh in range(2):
                                        nc.tensor.matmul(
                                            av_ps[2 * p + hh][:, 0:c.TOK],
                                            vt(kv, g),
                                            exm[:, hh * c.TOK:(hh + 1) * c.TOK],
                                            start=(g == 0),
                                            stop=(g == c.NKT - 1),
                                            skip_group_check=True)
                            else:
                                # compact: all 4 heads' B-blocks in [128,1024].
                                # Column map (hh%2)*TOK + (hh//2)*BLK puts the
                                # concurrently-executing row-tiled pairs
                                # (hh even at array rows 0-63, hh odd at
                                # 64-127) into different PSUM banks.
                                def bcol(hh):
                                    return (hh % 2) * c.TOK + (hh // 2) * c.BLK

                                sps = spool.tile([c.KT, 2 * c.TOK], F32,
                                                 tag="s")
                                for hh in range(c.GQ):
                                    h = kv * c.GQ + hh
                                    pcol = (h // 2) * c.TOK + c.BLK
                                    r0 = (hh % 2) * c.HD
                                    nc.tensor.matmul(
                                        sps[:, bcol(hh):bcol(hh) + c.BLK],
                                        kd(hh % 2, kv, g),
                                        qT_sb[r0:r0 + c.HD, pcol:pcol + c.BLK],
                                        start=True, stop=True)
                                ex = epool.tile([c.KT, 2 * c.TOK], BF16,
                                                tag="ex")
                                if "exp" in ab:
                                    nc.scalar.activation(
                                        ex[0:1, 0:2], sps[0:1, 0:2], AF.Exp,
                                        bias=0.0, scale=0.125)
                                else:
                                    nc.scalar.activation(
                                        ex[:], sps[:], AF.Exp, bias=0.0,
                                        scale=float(1.0 / np.sqrt(c.HD)))
                                if "mask" in ab:
                                    exm = ex
                                else:
                                    exm = epool.tile([c.KT, 2 * c.TOK], BF16,
                                                     tag="exm")
                                    for hh in range(2):
                                        mask_eng[hh].tensor_mul(
                                            exm[:, hh * c.TOK:(hh + 1) * c.TOK],
                                            ex[:, hh * c.TOK:(hh + 1) * c.TOK],
                                            mk)
                                for hh in range(c.GQ):
                                    nc.tensor.matmul(
                                        av_ps[hh][:, c.BLK:2 * c.BLK],
                                        vt(kv, g),
                                        exm[:, bcol(hh):bcol(hh) + c.BLK],
                                        start=False,
                                        stop=(g == c.NKT - 1),
                                        skip_group_check=True)

                        # ---- normalization for this kv group ----
                        if "norm" in ab:
                            continue
                        l4 = npool.tile([97, c.TOK], F32, tag="l4")
                        for hh in range(c.GQ):
                            nc.vector.tensor_copy(
                                l4[32 * hh:32 * hh + 1, :],
                                av_ps[hh][c.HD:c.HD + 1, :])
                        linv = npool.tile([97, c.TOK], F32, tag="linv")
                        for hh in range(c.GQ):
                            nc.vector.reciprocal(
                                linv[32 * hh:32 * hh + 1, :],
                                l4[32 * hh:32 * hh + 1, :])
                        for pp in range(c.GQ // 2):
                            bc_ps = spool.tile([c.KT, 2 * c.TOK], F32, tag="s")
                            for hh in range(2):
                                hq = 2 * pp + hh
                                nc.tensor.matmul(
                                    bc_ps[hh * c.HD:(hh + 1) * c.HD, 0:c.TOK],
                                    ones4[32 * hq:32 * hq + 1, :],
                                    linv[32 * hq:32 * hq + 1, :],
                                    start=True, stop=True,
                                    tile_position=(32 * hq, hh * c.HD))
                            bc_sb = npool.tile([128, c.TOK], F32,
                                               tag="bcs")
                            nc.vector.tensor_copy(bc_sb[:],
                                                  bc_ps[0:128, 0:c.TOK])
                            for hh in range(2):
                                h = kv * c.GQ + 2 * pp + hh
                                nc.vector.tensor_mul(
                                    yhead_ap(h),
                                    av_ps[2 * pp + hh][0:c.HD, :],
                                    bc_sb[hh * c.HD:(hh + 1) * c.HD, :])

                    if skip_attn:
                        nc.vector.memset(yT_sb[:], 1.0)

                # ---------------- out projection ----------------
                with tc.tile_pool(name="wopool", bufs=2) as wopool, \
                     tc.tile_pool(name="opspool", bufs=4, space="PSUM") as opsp, \
                     tc.tile_pool(name="osb", bufs=4) as osb:
                    if "outproj" in ab:
                        for tt in range(c.TOK // 128):
                            o_sb = osb.tile([128, c.DIM], F32, tag="ot")
                            nc.scalar.copy(o_sb[:, 0:c.TOK],
                                           yT_sb[:, 0:c.TOK])
                            nc.sync.dma_start(out[tt * 128:(tt + 1) * 128, :],
                                              o_sb[:])
                    else:
                        for oc in range(c.DIM // 512):
                            ps_tt = [opsp.tile([128, 512], F32, tag="o",
                                               name=f"ops_{_rep}_{oc}_{tt}")
                                     for tt in range(c.TOK // 128)]
                            # all 16 ct-chunks of this oc-block in one 2MB DMA
                            wo_sb = wopool.tile([128, c.NCT * 512], BF16,
                                                tag="wo")
                            nc.sync.dma_start(
                                wo_sb[:].rearrange("p (ct o) -> p ct o",
                                                   ct=c.NCT),
                                woT[:, oc * 512:(oc + 1) * 512].rearrange(
                                    "(ct p) o -> p ct o", p=128))
                            for ct in range(c.NCT):
                                for tt in range(c.TOK // 128):
                                    nc.tensor.matmul(
                                        ps_tt[tt][:],
                                        yT_sb[:, ct * c.TOK + tt * 128:
                                              ct * c.TOK + (tt + 1) * 128],
                                        wo_sb[:, ct * 512:(ct + 1) * 512],
                                        start=(ct == 0),
                                        stop=(ct == c.NCT - 1))
                            for tt in range(c.TOK // 128):
                                o_sb = osb.tile([128, 512], F32, tag="ot")
                                nc.scalar.copy(o_sb[:], ps_tt[tt][:])
                                nc.sync.dma_start(
                                    out[tt * 128:(tt + 1) * 128,
                                        oc * 512:(oc + 1) * 512], o_sb[:])

    return nc


# ---------------------------------------------------------------------------
# host side
# ---------------------------------------------------------------------------


def _rope_perm(n_heads, hd):
    p = []
    for h in range(n_heads):
        p.extend(h * hd + np.arange(0, hd, 2))
        p.extend(h * hd + np.arange(1, hd, 2))
    return np.array(p)


def _cos_sin(positions, hd, base):
    inv = 1.0 / base ** (np.arange(0, hd, 2, dtype=np.float64) / hd)
    fr = np.outer(inv, positions.astype(np.float64))
    return np.cos(fr).astype(np.float32), np.sin(fr).astype(np.float32)


def make_inputs(cfg: Cfg, x, Wq, Wk, Wv, Wo):
    c = cfg
    permq = _rope_perm(c.NH, c.HD)
    permk = _rope_perm(c.NKV, c.HD)
    wqT = np.ascontiguousarray(Wq[permq].T.astype(BF))
    wkT = np.ascontiguousarray(Wk[permk].T.astype(BF))
    wvT = np.ascontiguousarray(Wv.T.astype(BF))
    woT = np.ascontiguousarray(Wo.T.astype(BF))

    in_maps = []
    for core in range(c.NCORES):
        b, jA, jB = core_blocks(core)
        toks = np.concatenate([
            np.arange(jA * c.BLK, (jA + 1) * c.BLK),
            np.arange(jB * c.BLK, (jB + 1) * c.BLK)])
        xTc = np.ascontiguousarray(x[b, toks, :].T.astype(BF))
        cos, sin = _cos_sin(toks, c.HD, c.rope_base)
        cosR = np.concatenate([cos, cos, cos, cos], axis=0).astype(BF)
        sinR = np.concatenate([-sin, sin, -sin, sin], axis=0).astype(BF)
        # mask2[kk, g*TOK + :]:
        #   g < 8 : [maskA_g (256) | ones (256)]  (B-half fully causal)
        #   g >= 8: [maskB_g (256) | maskB_g (256)]
        kk = np.arange(c.KT)
        m = np.empty((c.KT, c.NKT * c.TOK), dtype=BF)
        toksA, toksB = toks[0:c.BLK], toks[c.BLK:2 * c.BLK]
        for g in range(c.NKT):
            kpos = g * c.KT + kk
            if g < c.NKT // 2:
                mA = (kpos[:, None] <= toksA[None, :]).astype(BF)
                m[:, g * c.TOK:g * c.TOK + c.BLK] = mA
                m[:, g * c.TOK + c.BLK:(g + 1) * c.TOK] = BF(1.0)
            else:
                mB = (kpos[:, None] <= toksB[None, :]).astype(BF)
                m[:, g * c.TOK:g * c.TOK + c.BLK] = mB
                m[:, g * c.TOK + c.BLK:(g + 1) * c.TOK] = mB
        in_maps.append({
            "xT": xTc, "wqT": wqT, "wkT": wkT, "wvT": wvT, "woT": woT,
            "cosR": cosR, "sinR": sinR, "masks": m,
        })
    return in_maps


def assemble(cfg: Cfg, results):
    c = cfg
    out = np.empty((c.B, c.T, c.DIM), np.float32)
    for core in range(c.NCORES):
        b, jA, jB = core_blocks(core)
        o = results[core]["out"]
        out[b, jA * c.BLK:(jA + 1) * c.BLK] = o[0:c.BLK]
        out[b, jB * c.BLK:(jB + 1) * c.BLK] = o[c.BLK:2 * c.BLK]
    return out


_CACHE = {}


def kernel(x, Wq, Wk, Wv, Wo):
    _install_birfix()
    import os
    from concourse.bass_utils import run_bass_kernel_spmd

    cfg = FULL
    if "nc" not in _CACHE:
        _CACHE["nc"] = build_nc(cfg)
    nc = _CACHE["nc"]
    in_maps = make_inputs(cfg, np.asarray(x), np.asarray(Wq), np.asarray(Wk),
                          np.asarray(Wv), np.asarray(Wo))
    try:
        res = run_bass_kernel_spmd(nc, in_maps,
                                   core_ids=list(range(cfg.NCORES)))
    except ModuleNotFoundError:
        # BASS_TRACE=1 under axon needs the NTFF hook (antenv.axon_hooks),
        # which some environments lack. Retry untraced.
        os.environ["BASS_NEVER_TRACE"] = "1"
        res = run_bass_kernel_spmd(nc, in_maps,
                                   core_ids=list(range(cfg.NCORES)))
    return assemble(cfg, res.results)
